# revision 51
# baseline (speedup 1.0000x reference)
"""CAM-GAT layer kernel for 8 Trainium2 NeuronCores (Bass/Tile).

Reference math (per graph of N=21 joints, F=128 feats):
    h = x @ W1                         [N, F]
    s = h @ a1 ; t = h @ a2            [N]
    e[i,j] = leaky_relu(s_i + t_j, 0.2)
    beta = softmax_j(e)
    alpha = cam * beta
    x_agg = alpha @ h
    out = elu(concat([x_agg, x], -1) @ W2_w + W2_b)

Sharding: pure data parallelism over graphs; each of the 8 cores gets
B/8 = 2048 graphs (43008 rows); weights replicated.

Key algebraic restructuring vs the straightforward flow:
    x_agg @ W2a = (alpha @ (x @ W1)) @ W2a = (alpha @ x) @ (W1 @ W2a)
so h is never materialized: per chunk m1 = x_c^T-weighted alpha^T
(m1[f,i] = sum_j x[j,f] alpha[i,j]) and one constant-stationary matmul
with W12 = W1 @ W2a finishes the aggregation half.  s and t come from
xT via wa1/wa2 = W1 @ a1/2.  The output is produced TRANSPOSED
(out_d[f, rows]) so W2_b is a per-partition ACT bias and the final
matmuls are two big constant-stationary N=504 streams; the host
transposes back during unsharding.

Per-core dataflow (supertile = 24 graphs = 504 rows = 4 chunks x 126
rows; a chunk = 6 graphs of 21 rows on 126 partitions):
  x_bf  : one cast-DMA load  [126, 4, F] bf16
  xT    : PE transpose per chunk -> PSUM bf16 -> SBUF (DVE copy)
  s,t   : spread matmuls from xT (waS/waT stationaries)
  e     : one matmul e[i,(c,j)] = s_i + t_j + MNEG*offblock via L36/R36
  u     : leaky-relu on DVE: (e*0.2) max e         -> bf16
  E     : one ACT Exp [126,504] bf16
  Z     : DVE tensor_reduce per chunk -> rowsum[126,4]; reciprocal
  A     : gpsimd stt (E * rinv_c) * camBD_c        -> bf16
  at    : PE transpose A per chunk -> PSUM -> SBUF
  m1    : matmul per chunk (lhsT=x_bf_c, rhs=at_c) -> PSUM -> bf16
  oT    : matmul(W12, m1) + matmul(w2bb, xT)       -> PSUM [F, 504]
  elu   : em = ACT Exp(oT+bias); zr = ACT Relu(oT+bias);
          ot = gpsimd stt (em-1) min zr; one DMA store [F, 504]
"""

import sys

import numpy as np

try:
    import concourse  # noqa: F401
except ImportError:  # pragma: no cover
    sys.path.insert(0, "/opt/trn_rl_repo")

import ml_dtypes
import concourse.bass as bass
import concourse.bacc as bacc
import concourse.tile as tile
from concourse import mybir

FP32 = mybir.dt.float32
BF16 = mybir.dt.bfloat16
AF = mybir.ActivationFunctionType
ALU = mybir.AluOpType

N_JOINTS = 21
F = 128
B_TOTAL = 16384
N_CORES = 8
B_CORE = B_TOTAL // N_CORES            # 2048 graphs per core
ROWS_CORE = B_CORE * N_JOINTS          # 43008 rows per core

G_CHUNK = 6                            # graphs per chunk
RC = G_CHUNK * N_JOINTS                # 126 rows per chunk
NCH = 4                                # chunks per (full) supertile
ROWS_SUPER = NCH * RC                  # 504
MNEG = -60000.0                        # exp(0.2*MNEG) == 0 in fp32


def _row_plan(rows):
    plan = []
    r = 0
    while r < rows:
        st = min(ROWS_SUPER, rows - r)
        assert st % N_JOINTS == 0
        chunks = []
        c = 0
        while c < st:
            chunks.append(min(RC, st - c))
            c += RC
        plan.append(chunks)
        r += st
    return plan


def host_consts(cam, W1, a, W2_w, W2_b):
    """Precompute tiny replicated tensors on the host (numpy)."""
    cam = np.asarray(cam, np.float32)
    W1 = np.asarray(W1, np.float32)
    a = np.asarray(a, np.float32)
    W2_w = np.asarray(W2_w, np.float32)
    W2_b = np.asarray(W2_b, np.float32)
    wa1 = W1 @ a[:F]                   # [128]
    wa2 = W1 @ a[F:]                   # [128]
    W12 = W1 @ W2_w[:F]                # [128,128]  (x_agg @ W2a fused)

    # waS[c]: [128, 4], col c = wa1 (s-dense accumulating matmuls)
    waS = np.zeros((NCH, F, NCH), np.float32)
    for c in range(NCH):
        waS[c, :, c] = wa1

    blk = np.arange(RC) // N_JOINTS
    # e-matmul operands, contraction k in [0, 36):
    #   k 0-3  : L dynamic s rows      | R static delta(c==c') ones
    #   k 4-9  : L MNEG*(blk(i)==q)    | R (1 - (blk(j)==q)) tiled
    #   k 10-31: zeros (partition-alignment filler)
    #   k 32   : L ones                | R dynamic t row
    L36 = np.zeros((36, RC), np.float32)
    for q in range(G_CHUNK):
        L36[4 + q, :] = MNEG * (blk == q)
    L36[32, :] = 1.0

    R36 = np.zeros((36, ROWS_SUPER), np.float32)
    for c in range(NCH):
        R36[c, c * RC:(c + 1) * RC] = 1.0
    for q in range(G_CHUNK):
        pat = (blk != q).astype(np.float32)
        for c in range(NCH):
            R36[4 + q, c * RC:(c + 1) * RC] = pat

    # Transposed block-diag cam: camT[j, c*126+i] = cam[i%21, j%21] if
    # i//21 == j//21 else 0  (multiplies alphaT during the PSUM->SBUF move)
    camtblk = np.zeros((RC, RC), np.float32)
    for q in range(G_CHUNK):
        camtblk[q * N_JOINTS:(q + 1) * N_JOINTS,
                q * N_JOINTS:(q + 1) * N_JOINTS] = cam.T
    camT = np.tile(camtblk, (1, NCH))   # [126, 504]

    bf = ml_dtypes.bfloat16
    return {
        "w12b": W12.astype(bf),                      # [128,128]
        "w2bb": W2_w[F:].astype(bf),                 # [128,128]
        "was_": waS.astype(bf),                      # [4,128,4]
        "wa2c": wa2.reshape(F, 1).astype(bf),        # [128,1]
        "l36": L36.astype(bf),                       # [36,126]
        "r36": R36.astype(bf),                       # [36,504]
        "camt": camT.astype(bf),                     # [126,504]
        "biasc": W2_b.reshape(F, 1).astype(np.float32),  # [128,1]
        "ident": np.eye(RC, dtype=bf),               # [126,126]
        "idnt8": np.eye(F, dtype=bf),                # [128,128]
    }


CONST_SPECS = {
    "w12b": ([F, F], BF16),
    "w2bb": ([F, F], BF16),
    "was_": ([NCH, F, NCH], BF16),
    "wa2c": ([F, 1], BF16),
    "l36": ([36, RC], BF16),
    "r36": ([36, ROWS_SUPER], BF16),
    "camt": ([RC, ROWS_SUPER], BF16),
    "biasc": ([F, 1], FP32),
    "ident": ([RC, RC], BF16),
    "idnt8": ([F, F], BF16),
}


def build_program(rows=ROWS_CORE):
    nc = bacc.Bacc("TRN2", target_bir_lowering=False, debug=False,
                   enable_asserts=False)
    x_d = nc.dram_tensor("x", [rows, F], FP32, kind="ExternalInput").ap()
    # transposed output: [F, rows]; host transposes back
    out_d = nc.dram_tensor("out", [F, rows], FP32, kind="ExternalOutput").ap()
    cst = {k: nc.dram_tensor(k, shape, dt, kind="ExternalInput").ap()
           for k, (shape, dt) in CONST_SPECS.items()}
    with tile.TileContext(nc) as tc:
        _body(tc, x_d, out_d, cst, rows)
    nc.compile()
    return nc


def _split_free(ap, n, inner):
    """View a 2D AP [P, n*inner] as 3D [P, n, inner]."""
    p, rest = ap.ap[0], list(ap.ap[1:])
    assert len(rest) == 1 and rest[0][0] == 1
    return bass.AP(ap.tensor, ap.offset, [p, [inner, n], [1, inner]])


def _body(tc, x_d, out_d, cst, rows):
    from contextlib import ExitStack
    nc = tc.nc
    plan = _row_plan(rows)

    with ExitStack() as ctx:
        # ---- pools ----
        cpool = ctx.enter_context(tc.tile_pool(name="consts", bufs=1))
        pin = ctx.enter_context(tc.tile_pool(name="xin", bufs=12))
        pxt = ctx.enter_context(tc.tile_pool(name="xt", bufs=12))
        pe_ = ctx.enter_context(tc.tile_pool(name="esb", bufs=8))
        psc = ctx.enter_context(tc.tile_pool(name="scal", bufs=8))
        pa = ctx.enter_context(tc.tile_pool(name="alpha", bufs=8))
        pat = ctx.enter_context(tc.tile_pool(name="alphat", bufs=8))
        pm1 = ctx.enter_context(tc.tile_pool(name="m1sb", bufs=8))
        pem = ctx.enter_context(tc.tile_pool(name="embuf", bufs=8))
        pzr = ctx.enter_context(tc.tile_pool(name="zrbuf", bufs=8))
        pout = ctx.enter_context(tc.tile_pool(name="outsb", bufs=8))

        ps_xt = ctx.enter_context(tc.tile_pool(name="ps_xt", bufs=1, space="PSUM"))
        ps_st = ctx.enter_context(tc.tile_pool(name="ps_st", bufs=1, space="PSUM"))
        ps_e = ctx.enter_context(tc.tile_pool(name="ps_e", bufs=3, space="PSUM"))
        ps_at = ctx.enter_context(tc.tile_pool(name="ps_at", bufs=1, space="PSUM"))
        ps_m1 = ctx.enter_context(tc.tile_pool(name="ps_m1", bufs=1, space="PSUM"))
        ps_o = ctx.enter_context(tc.tile_pool(name="ps_o", bufs=1, space="PSUM"))

        # ---- load constants ----
        w12b = cpool.tile([F, F], BF16, tag="w12b")
        w2bb = cpool.tile([F, F], BF16, tag="w2bb")
        was_ = cpool.tile([F, NCH, NCH], BF16, tag="was_")
        wa2c = cpool.tile([F, 1], BF16, tag="wa2c")
        camt = cpool.tile([RC, ROWS_SUPER], BF16, tag="camt")
        biasc = cpool.tile([F, 1], FP32, tag="biasc")
        ident = cpool.tile([RC, RC], BF16, tag="ident")
        idnt8 = cpool.tile([F, F], BF16, tag="idnt8")
        nc.sync.dma_start(idnt8[:], cst["idnt8"][:])
        nc.sync.dma_start(w12b[:], cst["w12b"][:])
        nc.sync.dma_start(w2bb[:], cst["w2bb"][:])
        nc.sync.dma_start(was_[:], cst["was_"].rearrange("c f e -> f c e"))
        nc.sync.dma_start(wa2c[:], cst["wa2c"][:])
        nc.sync.dma_start(camt[:], cst["camt"][:])
        nc.sync.dma_start(biasc[:], cst["biasc"][:])
        nc.sync.dma_start(ident[:], cst["ident"][:])

        # L/R e-matmul tiles (even/odd persistent): dynamic rows 0-3 (s) on
        # L and row 32 (t) on R; everything else static
        LRs = []
        for par in ("ev", "od"):
            Lt = cpool.tile([36, RC], BF16, tag=f"L_{par}")
            Rt = cpool.tile([36, ROWS_SUPER], BF16, tag=f"R_{par}")
            nc.sync.dma_start(Lt[:], cst["l36"][:])
            nc.sync.dma_start(Rt[:], cst["r36"][:])
            LRs.append((Lt, Rt))

        # Software-pipelined emission: A(n+1) is issued before B(n) so no
        # engine's FIFO head-of-line blocks on the softmax chain; C lags two.
        offs = []
        r0 = 0
        for chunks in plan:
            offs.append(r0)
            r0 += sum(chunks)

        def stage_a(sti):
            chunks = plan[sti]
            nch = len(chunks)
            st_rows = sum(chunks)
            full = st_rows == ROWS_SUPER
            r0 = offs[sti]
            Lt, Rt = LRs[sti % 2]

            # load x (f32 -> bf16 cast DMA on gpsimd)
            x_bf = pin.tile([RC, NCH, F], BF16, tag="x_bf")
            if full:
                nc.gpsimd.dma_start(
                    x_bf[:],
                    x_d[r0:r0 + ROWS_SUPER, :].rearrange(
                        "(c j) f -> j c f", j=RC))
            else:
                for c in range(nch):
                    rc = chunks[c]
                    if rc < RC:
                        nc.gpsimd.memset(x_bf[:, c, :], 0.0)
                    nc.gpsimd.dma_start(
                        x_bf[0:rc, c, :],
                        x_d[r0 + c * RC:r0 + c * RC + rc, :])

            # transpose x chunks: xT[f, (c, j)] flat
            xt_ps = ps_xt.tile([F, ROWS_SUPER], BF16, tag="xt_ps")
            for c in range(nch):
                nc.tensor.transpose(xt_ps[:, c * RC:(c + 1) * RC],
                                    x_bf[:, c, :], ident[:])
            xt = pxt.tile([F, ROWS_SUPER], BF16, tag="xt")
            nc.vector.tensor_copy(xt[:, 0:nch * RC], xt_ps[:, 0:nch * RC])

            # s dense (rows 32:36, accum) + t single row 0, one PSUM bank
            st_ps = ps_st.tile([36, ROWS_SUPER], FP32, tag="st_ps")
            for c in range(nch):
                nc.tensor.matmul(st_ps[32:36, 0:RC], was_[:, c, :],
                                 xt[:, c * RC:(c + 1) * RC],
                                 start=(c == 0), stop=(c == nch - 1))
            nc.tensor.matmul(st_ps[0:1, 0:st_rows], wa2c[:],
                             xt[:, 0:st_rows], start=True, stop=True)
            nc.vector.tensor_copy(Lt[0:4, :], st_ps[32:36, 0:RC])
            nc.scalar.copy(Rt[32:33, 0:st_rows], st_ps[0:1, 0:st_rows])

            # e = s + t + mask (one matmul)
            e_ps = ps_e.tile([RC, ROWS_SUPER], FP32, tag="e_ps")
            nc.tensor.matmul(e_ps[:, 0:st_rows], Lt[:], Rt[:, 0:st_rows],
                             start=True, stop=True)

            # early half of the output matmul: only needs xT (PE filler
            # while the softmax chain runs)
            o_ps = ps_o.tile([F, ROWS_SUPER], FP32, tag="o_ps")
            nc.tensor.matmul(o_ps[:, 0:st_rows], w2bb[:], xt[:, 0:st_rows],
                             start=True, stop=False)
            return dict(chunks=chunks, nch=nch, st_rows=st_rows, full=full,
                        r0=r0, x_bf=x_bf, xt=xt, e_ps=e_ps, o_ps=o_ps)

        def stage_b(st):
            chunks, nch, st_rows, full = (st["chunks"], st["nch"],
                                          st["st_rows"], st["full"])
            x_bf, xt, e_ps = st["x_bf"], st["xt"], st["e_ps"]

            # E = exp(lrelu(e)) = max(exp(e), exp(0.2e))
            ex1 = pe_.tile([RC, ROWS_SUPER], BF16, tag="ex1")
            nc.scalar.activation(ex1[:, 0:st_rows], e_ps[:, 0:st_rows],
                                 AF.Exp)
            ex2 = pe_.tile([RC, ROWS_SUPER], BF16, tag="ex2")
            nc.scalar.activation(ex2[:, 0:st_rows], e_ps[:, 0:st_rows],
                                 AF.Exp, scale=0.2)
            E = pe_.tile([RC, ROWS_SUPER], BF16, tag="E")
            nc.vector.scalar_tensor_tensor(
                E[:, 0:st_rows], ex1[:, 0:st_rows], 1.0, ex2[:, 0:st_rows],
                op0=ALU.mult, op1=ALU.max)
            rowsum = psc.tile([RC, NCH], FP32, tag="rowsum")
            if full:
                nc.vector.tensor_reduce(rowsum[:],
                                        _split_free(E[:], NCH, RC),
                                        axis=mybir.AxisListType.X, op=ALU.add)
            else:
                for c in range(nch):
                    rc = chunks[c]
                    nc.vector.tensor_reduce(rowsum[:, c:c + 1],
                                            E[:, c * RC:c * RC + rc],
                                            axis=mybir.AxisListType.X,
                                            op=ALU.add)
            rinv = psc.tile([RC, NCH], FP32, tag="rinv")
            nc.vector.reciprocal(rinv[:, 0:nch], rowsum[:, 0:nch])

            # A = E * rinv (bcast over chunk cols)
            A = pa.tile([RC, ROWS_SUPER], BF16, tag="A")
            if full:
                rinv_b = bass.AP(rinv.tensor, rinv.offset,
                                 [rinv.ap[0], [1, NCH], [0, RC]])
                nc.vector.tensor_tensor(
                    _split_free(A[:], NCH, RC), _split_free(E[:], NCH, RC),
                    rinv_b, ALU.mult)
            else:
                for c in range(nch):
                    rc = chunks[c]
                    sl = slice(c * RC, c * RC + rc)
                    rv = rinv[:, c:c + 1]
                    rinv_c = bass.AP(rv.tensor, rv.offset,
                                     [rv.ap[0], [0, rc]])
                    nc.vector.tensor_tensor(A[:, sl], E[:, sl], rinv_c,
                                            ALU.mult)

            # alphaT via PE transpose; camT applied during PSUM->SBUF
            at_ps = ps_at.tile([RC, ROWS_SUPER], BF16, tag="at_ps")
            for c in range(nch):
                rc = chunks[c]
                nc.tensor.transpose(at_ps[0:rc, c * RC:(c + 1) * RC],
                                    A[:, c * RC:c * RC + rc], ident[:])
            at = pat.tile([RC, ROWS_SUPER], BF16, tag="at")
            if full:
                nc.vector.tensor_tensor(at[:], at_ps[:], camt[:], ALU.mult)
            else:
                for c in range(nch):
                    rc = chunks[c]
                    sl = slice(c * RC, c * RC + rc)
                    nc.vector.tensor_tensor(at[0:rc, sl], at_ps[0:rc, sl],
                                            camt[0:rc, sl], ALU.mult)

            # m1[f, (c,i)] = sum_j x[j,f] alpha[i,j]*cam[i,j]
            m1_ps = ps_m1.tile([F, ROWS_SUPER], FP32, tag="m1_ps")
            for c in range(nch):
                rc = chunks[c]
                nc.tensor.matmul(m1_ps[:, c * RC:c * RC + rc],
                                 x_bf[0:rc, c, :],
                                 at[0:rc, c * RC:c * RC + rc],
                                 start=True, stop=True)
            m1 = pm1.tile([F, ROWS_SUPER], BF16, tag="m1")
            nc.scalar.copy(m1[:, 0:st_rows], m1_ps[:, 0:st_rows])

            # finish oT accumulation: += W12^T @ m1   [fo, (c,i)]
            o_ps = st["o_ps"]
            nc.tensor.matmul(o_ps[:, 0:st_rows], w12b[:], m1[:, 0:st_rows],
                             start=False, stop=True)

            em = pem.tile([F, ROWS_SUPER], BF16, tag="em")
            nc.scalar.activation(em[:, 0:st_rows], o_ps[:, 0:st_rows],
                                 AF.Exp, bias=biasc[:, 0:1])
            zr = pzr.tile([F, ROWS_SUPER], BF16, tag="zr")
            nc.scalar.activation(zr[:, 0:st_rows], o_ps[:, 0:st_rows],
                                 AF.Relu, bias=biasc[:, 0:1])
            return (em, zr, st["r0"], st_rows)

        def stage_c(em, zr, t0, t_rows):
            # elu(z) = min(exp(z)-1, relu(z)); store on the sync queue so
            # it never head-of-line blocks the gpsimd load queue
            ot = pout.tile([F, ROWS_SUPER], FP32, tag="ot")
            nc.vector.scalar_tensor_tensor(
                ot[:, 0:t_rows], em[:, 0:t_rows], -1.0, zr[:, 0:t_rows],
                op0=ALU.add, op1=ALU.min)
            nc.sync.dma_start(out_d[:, t0:t0 + t_rows], ot[:, 0:t_rows])

        n_st = len(plan)
        pend = None
        for n in range(n_st):
            st = stage_a(n)
            if pend is not None:
                stage_c(*pend)
            pend = stage_b(st)
        stage_c(*pend)


# ---------------------------------------------------------------------------
_PROG_CACHE = {}


def _get_program(rows):
    if rows not in _PROG_CACHE:
        _PROG_CACHE[rows] = build_program(rows)
    return _PROG_CACHE[rows]


def kernel(x, cam, W1, a, W2_w, W2_b):
    from concourse.bass_utils import run_bass_kernel_spmd

    x = np.ascontiguousarray(np.asarray(x, np.float32))
    consts = host_consts(cam, W1, a, W2_w, W2_b)
    nc = _get_program(ROWS_CORE)

    in_maps = []
    for core in range(N_CORES):
        m = {"x": x[core * ROWS_CORE:(core + 1) * ROWS_CORE]}
        m.update(consts)
        in_maps.append(m)
    res = run_bass_kernel_spmd(nc, in_maps, list(range(N_CORES)))
    out = np.concatenate(
        [np.ascontiguousarray(res.results[i]["out"].T) for i in range(N_CORES)],
        axis=0)
    return out.astype(np.float32)
